# revision 8
# baseline (speedup 1.0000x reference)
"""Trainium2 Bass kernel for nn_CrossAttention (sparse cross-attention).

Math (reference):
    q = xF @ Wq;  k = context @ Wk;  v = context @ Wv
    attn = softmax(scale * q k^T) over K keys
    out = (attn v) @ Wo + bo + xF, rows >= lengths[b] zeroed

Algebraic restructure (context has only 4 channels, so per-head k/v are
rank-4):
    Wqk[ch, h, e] = scale * sum_d Wq[ch, hD+d] Wk[e, hD+d]
    sT_h[key,row] = ctx[key,:] . (Wqk_h^T xF_row)         (PE, contraction 4)
    avT[(h,e), row] = sum_k exp(sT_h) [ctx;1][k,e]        (PE, contraction 128)
    out = Wvo^T (avT / lam) + bo + xF,   Wvo_h = Wv_h Wo_h

Single-pass interleaved head layout: all 8 heads live in one 128-partition
tile.  qk components at partitions 32g+4p+e (head 2g+p); av/avn at
partitions 16h+e (e<4: value comps, e=4: softmax normalizer from a ones
column).  Score matmuls are 4-way row-tiled (strip g), av matmuls 4-way
col-tiled, using zero-padded context operand variants (A: head-even rows,
B: head-odd rows) so every operand stays 32-partition aligned.

The exp (8 heads x 256 keys per row) is the roofline: it runs on the
scalar engine at 1 elem/cycle/lane.  Everything else is scheduled to hide
behind it: scores are exp'd directly from PSUM in 6 groups per supertile
([3,3,3,3,2,2] strips) that ping-pong between two 3-bank PSUM slots while
qk/av/lam/out share the remaining 2 banks via phase-disjoint pools (qk
rides in the unused 3rd bank of the 5th score group).  The emission is
software-pipelined across supertiles so the scalar engine never waits.

All dataflow is f16 (inputs converted on host); PSUM accumulation stays
f32.  Per-core work: T=ceil(tiles/8) supertiles of R=512 rows (valid rows
packed per batch, spread over 8 cores).
"""

import numpy as np

NUM_HEAD = 8
CH_HEAD = 64
CH = 512
CONTEXT_CH = 4
B, L, K = 16, 4096, 256
R = 512
N_CORES = 8

# exp groups: (slot 'A'|'B', strip start, strip end)
GROUPS = [("A", 0, 3), ("B", 3, 6), ("A", 6, 9), ("B", 9, 12),
          ("A", 12, 14), ("B", 14, 16)]


def _strip_info(s):
    g, wave = s % 4, s // 4
    return g, wave // 2, wave % 2  # row/col strip, parity, key chunk


def _build_host_constants(Wq, Wk, Wv, Wo, bo):
    scale = CH_HEAD ** (-0.5)
    Wq_h = Wq.reshape(CH, NUM_HEAD, CH_HEAD)
    Wk_h = Wk.reshape(CONTEXT_CH, NUM_HEAD, CH_HEAD)
    Wqk = scale * np.einsum("chd,ehd->che", Wq_h, Wk_h)  # [512, 8, 4]

    wqk = np.zeros((128, 4, 128), np.float16)  # [ch_part, chunk, m]
    for g in range(4):
        for par in range(2):
            h = 2 * g + par
            for c in range(4):
                wqk[:, c, 32 * g + 4 * par: 32 * g + 4 * par + 4] = \
                    Wqk[128 * c: 128 * (c + 1), h, :]

    Wv_h = Wv.reshape(CONTEXT_CH, NUM_HEAD, CH_HEAD)
    Wo_h = Wo.reshape(NUM_HEAD, CH_HEAD, CH)
    wvo = np.zeros((128, CH), np.float16)
    for h in range(NUM_HEAD):
        wvo[16 * h: 16 * h + 4, :] = Wv_h[:, h, :] @ Wo_h[h]
        wvo[16 * h + 4, :] = bo / NUM_HEAD  # avn row 16h+4 == 1.0 exactly
    ssel = np.zeros((128, 128), np.float16)
    for g in range(4):
        for par in range(2):
            base = 32 * g + 16 * par
            ssel[base + 4, base: base + 16] = 1.0
    return wqk, wvo, ssel


def _build_context(context):
    ctx = np.zeros((B, 128, 2, 256), np.float16)   # [...][:128]=A, [128:]=B
    c5 = np.zeros((B, 128, 2, 64), np.float16)     # [...][:32]=A, [32:]=B
    for b in range(B):
        cT = context[b].T  # [4, 256]
        for g in range(4):
            for kc in range(2):
                ctx[b, 32 * g: 32 * g + 4, kc, 0:128] = cT[:, 128 * kc:128 * (kc + 1)]
                ctx[b, 32 * g + 4: 32 * g + 8, kc, 128:256] = cT[:, 128 * kc:128 * (kc + 1)]
        for kc in range(2):
            c5[b, :, kc, 0:4] = context[b, 128 * kc:128 * (kc + 1), :]
            c5[b, :, kc, 4] = 1.0
            c5[b, :, kc, 32 + 16:32 + 20] = context[b, 128 * kc:128 * (kc + 1), :]
            c5[b, :, kc, 32 + 20] = 1.0
    return ctx, c5


def _build_program(T):
    import concourse.bass as bass  # noqa: F401
    import concourse.tile as tile
    from concourse import bacc, mybir

    f32 = mybir.dt.float32
    f16 = mybir.dt.float16
    Exp = mybir.ActivationFunctionType.Exp

    nc = bacc.Bacc("TRN2", target_bir_lowering=False, debug=False)

    xft_d = nc.dram_tensor("xft", [T, 128, 4, R], f16, kind="ExternalInput").ap()
    ctx_d = nc.dram_tensor("ctx", [T, 128, 2, 256], f16, kind="ExternalInput").ap()
    c5_d = nc.dram_tensor("c5", [T, 128, 2, 64], f16, kind="ExternalInput").ap()
    wqk_d = nc.dram_tensor("wqk", [128, 4, 128], f16, kind="ExternalInput").ap()
    wvo_d = nc.dram_tensor("wvo", [128, CH], f16, kind="ExternalInput").ap()
    ssel_d = nc.dram_tensor("ssel", [128, 128], f16, kind="ExternalInput").ap()
    out_d = nc.dram_tensor("outt", [T, 128, 4, R], f16, kind="ExternalOutput").ap()

    with tile.TileContext(nc) as tc:
        consts = tc.alloc_tile_pool(name="consts", bufs=1)
        wqk_s = consts.tile([128, 4, 128], f16)
        wvo_s = consts.tile([128, CH], f16)
        ssel_s = consts.tile([128, 128], f16)
        expbias = consts.tile([128, 1], f32)
        nc.vector.memset(expbias, -4.0)
        nc.scalar.dma_start(out=wqk_s, in_=wqk_d)
        nc.scalar.dma_start(out=wvo_s, in_=wvo_d)
        nc.scalar.dma_start(out=ssel_s, in_=ssel_d)

        io = tc.alloc_tile_pool(name="io", bufs=3)
        ctxp = tc.alloc_tile_pool(name="ctxp", bufs=2)
        exp_pool = tc.alloc_tile_pool(name="exp_pool", bufs=2)
        workp = tc.alloc_tile_pool(name="workp", bufs=2)
        outsb = tc.alloc_tile_pool(name="outsb", bufs=2)
        ps_sc = tc.alloc_tile_pool(name="ps_sc", bufs=1, space="PSUM")
        ps_av = tc.alloc_tile_pool(name="ps_av", bufs=1, space="PSUM")
        ps_o = tc.alloc_tile_pool(name="ps_o", bufs=1, space="PSUM")

        # per-iteration live state
        xft = [None] * T
        ctxt = [None] * T
        c5t = [None] * T
        qk_sb = [None] * T
        ex = [None] * T
        av_ps = [None] * T
        avn = [None] * T
        out_sb = [None] * T
        sc_tiles = {}  # (t, group) -> psum tile

        def dma_in(t, eng=None):
            eng = eng or nc.sync
            xft[t] = io.tile([128, 4, R], f16, tag="xft", name="xft")
            eng.dma_start(out=xft[t], in_=xft_d[t])
            ctxt[t] = ctxp.tile([128, 2, 256], f16, tag="ctx", name="ctx")
            eng.dma_start(out=ctxt[t], in_=ctx_d[t])
            c5t[t] = ctxp.tile([128, 2, 64], f16, tag="c5", name="c5")
            eng.dma_start(out=c5t[t], in_=c5_d[t])

        def alloc_sc(t, k):
            slot = GROUPS[k][0]
            sc_tiles[(t, k)] = ps_sc.tile([128, 3, R], f32, tag=f"sc{slot}", name=f"sc{slot}")
            return sc_tiles[(t, k)]

        def emit_sc_group(t, k):
            """Score matmuls for exp group k of tile t."""
            sc = sc_tiles.get((t, k))
            if sc is None:
                sc = alloc_sc(t, k)
            _, s0, s1 = GROUPS[k]
            for i, s in enumerate(range(s0, s1)):
                g, par, kc = _strip_info(s)
                nc.tensor.matmul(
                    out=sc[:, i, :],
                    lhsT=ctxt[t][32 * g: 32 * g + 8, kc, 128 * par: 128 * (par + 1)],
                    rhs=qk_sb[t][32 * g: 32 * g + 8, :],
                    tile_position=(32 * g, 0),
                )

        def emit_exp(t, k):
            _, s0, s1 = GROUPS[k]
            n = s1 - s0
            nc.scalar.activation(
                out=ex[t][:, s0:s1, :],
                in_=sc_tiles[(t, k)][:, 0:n, :],
                func=Exp,
                bias=expbias,
            )

        def emit_qk(t, home):
            """qk projection for tile t into PSUM strip `home` ([128, R])."""
            for c in range(4):
                nc.tensor.matmul(
                    out=home,
                    lhsT=wqk_s[:, c, :],
                    rhs=xft[t][:, c, :],
                    start=(c == 0),
                    stop=(c == 3),
                )

        def emit_qk_cast(t, home):
            qk_sb[t] = workp.tile([128, R], f16, tag="qk_sb", name="qk_sb")
            nc.vector.tensor_copy(out=qk_sb[t], in_=home)

        def emit_av(t, s0, s1):
            if s0 == 0:
                av_ps[t] = ps_av.tile([128, R], f32, tag="avlam", name="av_ps")
            for s in range(s0, s1):
                g, par, kc = _strip_info(s)
                nc.tensor.matmul(
                    out=av_ps[t][32 * g: 32 * (g + 1), :],
                    lhsT=c5t[t][:, kc, 32 * par: 32 * par + 32],
                    rhs=ex[t][:, s, :],
                    tile_position=(0, 32 * g),
                    start=(s < 4),
                    stop=(s >= 12),
                )

        def emit_norm(t):
            av_sb = workp.tile([128, R], f32, tag="av_sb")
            nc.vector.tensor_copy(out=av_sb, in_=av_ps[t])
            lam_sb = workp.tile([128, R], f32, tag="lam_sb")
            nc.vector.stream_shuffle(out=lam_sb, in_=av_sb,
                                     mask=[4] * 16 + [20] * 16)
            lr_sb = workp.tile([128, R], f32, tag="lr_sb")
            nc.vector.reciprocal_approx_fast(out=lr_sb, in_=lam_sb)
            avn[t] = workp.tile([128, R], f16, tag="avn", name="avn")
            nc.gpsimd.tensor_mul(avn[t], av_sb, lr_sb)

        def emit_out_chunk(t, c):
            if c == 0:
                out_sb[t] = outsb.tile([128, 4, R], f16, tag="out_sb",
                                       name="out_sb")
            o_ps = ps_o.tile([128, R], f32, tag="o", name="o_ps")
            nc.tensor.matmul(
                out=o_ps,
                lhsT=wvo_s[:, 128 * c: 128 * (c + 1)],
                rhs=avn[t],
            )
            nc.vector.tensor_add(out_sb[t][:, c, :], o_ps, xft[t][:, c, :])
            if c == 3:
                nc.sync.dma_start(out=out_d[t], in_=out_sb[t])

        # ---- prologue ----
        dma_in(0)
        if T > 1:
            dma_in(1, eng=nc.gpsimd)
        pre = ps_sc.tile([128, 3, R], f32, tag="scA")
        emit_qk(0, pre[:, 2, :])
        emit_qk_cast(0, pre[:, 2, :])

        # ---- main loop ----
        for t in range(T):
            ex[t] = exp_pool.tile([128, 16, R], f16, tag="ex", name="ex")
            if t == 0:
                alloc_sc(0, 0)
                alloc_sc(0, 1)
                emit_sc_group(0, 0)
                emit_sc_group(0, 1)
            emit_exp(t, 0)
            emit_exp(t, 1)
            emit_sc_group(t, 2)
            emit_exp(t, 2)
            emit_sc_group(t, 3)
            emit_exp(t, 3)
            emit_av(t, 0, 6)
            g4 = alloc_sc(t, 4)
            emit_sc_group(t, 4)
            emit_exp(t, 4)
            if t + 1 < T:
                emit_qk(t + 1, g4[:, 2, :])
                emit_qk_cast(t + 1, g4[:, 2, :])
            if t >= 1:
                emit_out_chunk(t - 1, 0)
            emit_sc_group(t, 5)
            emit_exp(t, 5)
            if t + 1 < T:
                alloc_sc(t + 1, 0)
                alloc_sc(t + 1, 1)
                emit_sc_group(t + 1, 0)
            emit_av(t, 6, 12)
            if t >= 1:
                emit_out_chunk(t - 1, 1)
            if t + 1 < T:
                emit_sc_group(t + 1, 1)
            emit_av(t, 12, 14)
            if t >= 1:
                emit_out_chunk(t - 1, 2)
            emit_av(t, 14, 16)
            emit_norm(t)
            if t >= 1:
                emit_out_chunk(t - 1, 3)
            if t + 2 < T:
                dma_in(t + 2)

        for c in range(4):
            emit_out_chunk(T - 1, c)

        for pool in (ps_o, ps_av, ps_sc, outsb, workp, exp_pool, ctxp, io,
                     consts):
            pool.release()

    nc.compile()
    return nc


def _plan_supertiles(lengths):
    """Split each batch's valid rows into R-row supertiles; spread over cores."""
    tiles = []  # (batch, row0, nvalid)
    for b in range(B):
        nb = int(lengths[b])
        r0 = 0
        while r0 < nb:
            tiles.append((b, r0, min(R, nb - r0)))
            r0 += R
    T = max(1, (len(tiles) + N_CORES - 1) // N_CORES)
    per_core = [tiles[c * T: (c + 1) * T] for c in range(N_CORES)]
    return per_core, T


def kernel(xF, context, lengths, Wq, Wk, Wv, Wo, bo):
    from concourse import bass_utils

    xF = np.asarray(xF, np.float32)
    context = np.asarray(context, np.float32)
    lengths_np = np.asarray(lengths, np.int32)

    wqk, wvo, ssel = _build_host_constants(
        np.asarray(Wq, np.float32),
        np.asarray(Wk, np.float32),
        np.asarray(Wv, np.float32),
        np.asarray(Wo, np.float32),
        np.asarray(bo, np.float32),
    )
    ctx_b, c5_b = _build_context(context)

    per_core, T = _plan_supertiles(lengths_np)
    nc = _build_program(T)

    in_maps = []
    for c in range(N_CORES):
        xft = np.zeros((T, 128, 4, R), np.float16)
        ctx = np.zeros((T, 128, 2, 256), np.float16)
        c5 = np.zeros((T, 128, 2, 64), np.float16)
        c5[:, :, :, 4] = 1.0  # dummy tiles: finite normalizer
        c5[:, :, :, 32 + 20] = 1.0
        for t, (b, r0, nv) in enumerate(per_core[c]):
            blockT = np.zeros((CH, R), np.float32)
            blockT[:, :nv] = xF[b, r0: r0 + nv, :].T
            xft[t] = blockT.reshape(4, 128, R).transpose(1, 0, 2)
            ctx[t] = ctx_b[b]
            c5[t] = c5_b[b]
        in_maps.append(
            {
                "xft": xft,
                "ctx": ctx,
                "c5": c5,
                "wqk": wqk,
                "wvo": wvo,
                "ssel": ssel,
            }
        )

    import os

    trace = bool(os.environ.get("CA_TRACE"))
    res = bass_utils.run_bass_kernel_spmd(
        nc,
        in_maps,
        core_ids=list(range(N_CORES)),
        trace=trace,
        **({"tmpdir": "/tmp/ca_prof"} if trace else {}),
    )
    if trace and res.exec_time_ns is not None:
        print(f"HW exec time: {res.exec_time_ns} ns")

    out = np.zeros((B, L, CH), np.float32)
    for c in range(N_CORES):
        arr = np.asarray(res.results[c]["outt"], np.float32)  # [T, 128, 4, R]
        for t, (b, r0, nv) in enumerate(per_core[c]):
            rows = arr[t].transpose(2, 1, 0).reshape(R, CH)  # [row, ch]
            out[b, r0: r0 + nv, :] = rows[:nv]
    return out
